# revision 65
# baseline (speedup 1.0000x reference)
import sys

import numpy as np

for p in ("/opt/trn_rl_repo",):
    if p not in sys.path:
        sys.path.insert(0, p)

import ml_dtypes  # noqa: E402

import concourse.tile as tile  # noqa: E402
from concourse import bacc, mybir  # noqa: E402
from concourse.bass_utils import run_bass_kernel_spmd  # noqa: E402

B, N, D = 128, 512, 512
NCORES = 8
BPC = B // NCORES  # 16 batch items per core
F32 = mybir.dt.float32
BF16 = mybir.dt.bfloat16
ACT_COPY = mybir.ActivationFunctionType.Copy


def _hadamard(n: int) -> np.ndarray:
    H = np.array([[1.0]], dtype=np.float32)
    base = np.array([[1.0, 1.0], [1.0, -1.0]], dtype=np.float32)
    while H.shape[0] < n:
        H = np.kron(H, base)
    return H


def _build():
    # y = H512 @ x @ H512 / 512 per item, via H512 = H2 (x) H256, with bf16
    # device I/O (the host converts; the 2e-2 tolerance has ~6x margin).
    #
    # Per item: H2 input butterfly over the 256-row halves (GpSimd sum, DVE
    # difference, bf16).  The rest of the pipeline splits into two fully
    # independent half-items (the input-side H2 index i selects disjoint
    # n-columns of t and disjoint n-rows of y), each using 4 PSUM banks:
    # stage B (8 matmuls vs H256, K=256 as two accumulating K=128 passes)
    # leaves t transposed in PSUM; Act evicts the d-high half (the DVE
    # crossing may read at most one PSUM operand); the DVE crossing applies
    # the output-side H2 butterfly; stage C (8 matmuls vs H256/512) produces
    # y rows in natural layout; Act evicts to bf16 and SP stores.
    # Half-items alternate between two 4-bank PSUM sets (bufs=2), and stage
    # C trails stage B by one half-item, so the PE never waits on the
    # crossing.
    nc = bacc.Bacc("TRN2", target_bir_lowering=False, debug=False)
    x_d = nc.dram_tensor("x", [BPC, 4, 128, D], BF16, kind="ExternalInput").ap()
    hc_d = nc.dram_tensor("hc", [128, 2, 2, 256], BF16, kind="ExternalInput").ap()
    y_d = nc.dram_tensor("y", [BPC, 4, 128, D], BF16, kind="ExternalOutput").ap()

    with tile.TileContext(nc) as tc:
        with (
            tc.tile_pool(name="const", bufs=1) as cpool,
            tc.tile_pool(name="xp", bufs=8) as xpool,
            tc.tile_pool(name="xc", bufs=4) as xcpool,
            tc.tile_pool(name="tp", bufs=5) as ttpool,
            tc.tile_pool(name="tb", bufs=4) as tbpool,
            tc.tile_pool(name="yp", bufs=5) as ypool,
            tc.tile_pool(name="ps1", bufs=2, space="PSUM") as ps1pool,
            tc.tile_pool(name="ps2", bufs=2, space="PSUM") as ps2pool,
        ):
            hc = cpool.tile([128, 2, 2, 256], BF16)
            h256 = hc[:, 0]  # [128, 2, 256]: rows of H256, split in halves
            hs256 = hc[:, 1]  # H256 / 512

            def stage_in(b):
                xt = xpool.tile([128, 4, D], BF16, tag="xt", name="xt")
                xcb = xcpool.tile([128, 2, 2, D], BF16, tag="xcb", name="xcb")
                xsrc = x_d[b].transpose([1, 0, 2])
                if b == 0:
                    # Pipeline head: item 0 loads and butterflies in column
                    # halves on the fast DVE, ordered to feed stage B's first
                    # matmuls (dt 2,3 read columns 256:512), with the
                    # constants DMA slotted between the halves.
                    hi, lo = slice(256, 512), slice(0, 256)
                    nc.sync.dma_start(xt[:, :, hi], xsrc[:, :, hi])
                    nc.sync.dma_start(hc[:], hc_d[:])
                    nc.sync.dma_start(xt[:, :, lo], xsrc[:, :, lo])
                    for sl in (hi, lo):
                        nc.vector.tensor_add(
                            xcb[:, 0, :, sl], xt[:, 0:2, sl], xt[:, 2:4, sl]
                        )
                    for sl in (hi, lo):
                        nc.vector.tensor_sub(
                            xcb[:, 1, :, sl], xt[:, 0:2, sl], xt[:, 2:4, sl]
                        )
                    return xcb
                nc.sync.dma_start(xt[:], xsrc)
                # Input-side H2 butterfly: xcb[:, i, h] = xt[:, h] +/- xt[:, 2+h]
                nc.gpsimd.tensor_add(xcb[:, 0], xt[:, 0:2], xt[:, 2:4])
                nc.vector.tensor_sub(xcb[:, 1], xt[:, 0:2], xt[:, 2:4])
                return xcb

            def stage_b_cross(xcb, i):
                # Stage B for half i: ps1[dt][d', q] = t[(i,q), dt*128+d']
                #        = sum_h xcb_i[:, h, dt-chunk].T @ H256[h-half]
                ps1a = ps1pool.tile([128, 2, 256], F32, tag="ps1a", name="ps1a")
                ps1b = ps1pool.tile([128, 2, 256], F32, tag="ps1b", name="ps1b")
                for dt in (2, 3, 0, 1):
                    bank = ps1a if dt < 2 else ps1b
                    for h in range(2):
                        nc.tensor.matmul(
                            bank[:, dt % 2],
                            xcb[:, i, h, dt * 128 : (dt + 1) * 128],
                            h256[:, h],
                            start=(h == 0),
                            stop=(h == 1),
                        )
                tb = tbpool.tile([128, 2, 256], BF16, tag="tb", name="tb")
                nc.scalar.activation(tb[:], ps1b[:], ACT_COPY)
                # PSUM crossing (DVE): output-side H2 butterfly over d-halves.
                ttc = ttpool.tile([128, 4, 256], BF16, tag="ttc", name="ttc")
                nc.vector.tensor_add(ttc[:, 0:2], ps1a[:], tb[:])
                nc.vector.tensor_sub(ttc[:, 2:4], ps1a[:], tb[:])
                return ttc

            def stage_c_out(b, i, ttc, yt, fine=False):
                # Stage C for half i (y rows 2i*128 .. (2i+2)*128):
                # ps2[:, nt2, ie*256+e'] =
                #   sum_dt' ttc[:, 2*ie+dt', nt2-chunk].T @ (H256/512)[dt'-half]
                bank = ps2pool.tile([128, 2, D], F32, tag="ps2", name="ps2")
                for nt2 in range(2):
                    for ie in range(2):
                        for dtp in range(2):
                            nc.tensor.matmul(
                                bank[:, nt2, ie * 256 : (ie + 1) * 256],
                                ttc[:, 2 * ie + dtp, nt2 * 128 : (nt2 + 1) * 128],
                                hs256[:, dtp],
                                start=(dtp == 0),
                                stop=(dtp == 1),
                            )
                if fine:
                    # Last half-item: evict+store per quarter so the final
                    # store overlaps the final eviction.
                    for nt2 in range(2):
                        nt = 2 * i + nt2
                        nc.scalar.activation(yt[:, nt], bank[:, nt2], ACT_COPY)
                        nc.sync.dma_start(y_d[b, nt], yt[:, nt])
                else:
                    nc.scalar.activation(
                        yt[:, 2 * i : 2 * i + 2], bank[:], ACT_COPY
                    )
                    nc.sync.dma_start(
                        y_d[b, 2 * i : 2 * i + 2].transpose([1, 0, 2]),
                        yt[:, 2 * i : 2 * i + 2],
                    )

            # Software pipeline over half-items: stage C trails by two, so
            # the crossing feeding each C finished well before the PE reaches
            # it — the PE queue never stalls (stalls also reset its clock
            # ramp).
            from collections import deque

            pending = deque()  # (b, i, ttc, yt)
            yts = {}
            for b in range(BPC):
                xcb = stage_in(b)
                yts[b] = ypool.tile([128, 4, D], BF16, tag="yt", name="yt")
                for i in range(2):
                    ttc = stage_b_cross(xcb, i)
                    pending.append((b, i, ttc, yts[b]))
                    if len(pending) > 4:
                        stage_c_out(*pending.popleft())
                if b - 2 in yts:
                    del yts[b - 2]
            while pending:
                stage_c_out(*pending.popleft(), fine=len(pending) == 0)

    nc.compile()
    return nc


_NC = None


def kernel(x: np.ndarray) -> np.ndarray:
    global _NC
    if _NC is None:
        _NC = _build()
    x = np.ascontiguousarray(
        np.asarray(x, dtype=np.float32).astype(ml_dtypes.bfloat16)
    )
    H = _hadamard(256)
    # hc[p, 0, h, q] = H256[h*128+p, q]; hc[p, 1, h, q] = H256[h*128+p, q]/512
    hrows = H.reshape(2, 128, 256).transpose(1, 0, 2)  # [128, 2, 256]
    hc = np.stack([hrows, hrows / np.float32(512.0)], axis=1)  # [128, 2, 2, 256]
    hc = np.ascontiguousarray(hc.astype(ml_dtypes.bfloat16))
    xr = x.reshape(NCORES, BPC, 4, 128, D)
    in_maps = [{"x": xr[i], "hc": hc} for i in range(NCORES)]
    res = run_bass_kernel_spmd(_NC, in_maps, list(range(NCORES))).results
    return np.concatenate(
        [np.asarray(r["y"]).reshape(BPC, N, D) for r in res], axis=0
    ).astype(np.float32)
